# revision 50
# baseline (speedup 1.0000x reference)
"""Bloom attention (separated QKV) — 8-core TRN2 Bass kernel.

Distribution: tensor-parallel over heads (2 heads/core). Each core:
  1. QKV projections for its 256-row slice of Wq/Wk/Wv (q^T,k^T in [d,s]
     layout, v in [s,d] layout, all bf16 in SBUF, fp32 accumulate).
  2. Attention with transposed scores St[k,q] = k @ q^T computed in
     qq=1024 groups, exp via ScalarE (alibi as per-partition bias),
     softmax denominator via ones-matmul, ctx^T = v^T @ P in PSUM,
     normalized by broadcast 1/den.
  3. Chunked AllGather (4 chunks along the sequence) of ctx^T slices
     (bf16), overlapped with the remaining attention blocks.
  4. Output projection for its 256-column slice of Wd + bias + residual,
     per gathered chunk.
Host side: transpose/slice/cast weights + hs (layout prep only),
concatenate the 8 output column-slices.
"""
import numpy as np
import ml_dtypes

import concourse.bass as bass
import concourse.bacc as bacc
import concourse.mybir as mybir
import concourse.tile as tile
import concourse.bass_utils as bass_utils

BF16 = ml_dtypes.bfloat16
N_CORES = 8
B, S, H = 2, 2048, 2048
NH, HD = 16, 128
HPC = NH // N_CORES          # heads per core
CI = HPC * HD                # per-core slice of H (256)
BS = B * S                   # 4096
INV_NORM = 1.0 / float(np.sqrt(HD))

JT = H // 128                # 16 contraction tiles for projections
SS_CHUNK = 512               # seq chunk for projections
N_CHUNKS = BS // SS_CHUNK    # 8
KT = S // 128                # 16 key tiles per batch
IT = H // 128                # 16 contraction tiles for dense
QBLK = 1024                  # attention/AG/dense block along seq
N_BLOCKS = BS // QBLK        # 4

F32 = mybir.dt.float32
BF = mybir.dt.bfloat16

DEBUG_OUTPUTS = False


def _build():
    nc = bacc.Bacc("TRN2", target_bir_lowering=False, debug=False,
                   num_devices=N_CORES)

    hsT = nc.dram_tensor("hsT", [H, BS], BF, kind="ExternalInput").ap()
    wqT = nc.dram_tensor("wqT", [H, CI], BF, kind="ExternalInput").ap()
    wkT = nc.dram_tensor("wkT", [H, CI], BF, kind="ExternalInput").ap()
    wvT = nc.dram_tensor("wvT", [H, CI], BF, kind="ExternalInput").ap()
    wdT = nc.dram_tensor("wdT", [H, CI], BF, kind="ExternalInput").ap()
    bq = nc.dram_tensor("bq", [CI, 1], F32, kind="ExternalInput").ap()
    bk = nc.dram_tensor("bk", [CI, 1], F32, kind="ExternalInput").ap()
    bv = nc.dram_tensor("bv", [1, CI], BF, kind="ExternalInput").ap()
    bd_f32 = nc.dram_tensor("bd", [CI, 1], F32, kind="ExternalInput").ap()
    alibi = nc.dram_tensor("alibi", [B * HPC, S], F32, kind="ExternalInput").ap()
    residT = nc.dram_tensor("residT", [CI, BS], F32, kind="ExternalInput").ap()
    outT = nc.dram_tensor("outT", [CI, BS], F32, kind="ExternalOutput").ap()

    bounce = nc.dram_tensor("bounce", [N_BLOCKS, HPC, 128, QBLK], BF,
                            kind="Internal").ap()
    # per-(block, hi) AllGather output: rows = core*128 + d
    gath = nc.dram_tensor("gath", [HPC, N_BLOCKS, N_CORES * 128, QBLK], BF,
                          kind="Internal", addr_space="Shared").ap()
    if DEBUG_OUTPUTS:
        qT_dbg = nc.dram_tensor("qT_dbg", [128, HPC * BS], BF,
                                kind="ExternalOutput").ap()
        kT_dbg = nc.dram_tensor("kT_dbg", [128, HPC * BS], BF,
                                kind="ExternalOutput").ap()
        v_dbg = nc.dram_tensor("v_dbg", [128, (BS // 128) * CI], BF,
                               kind="ExternalOutput").ap()
        ctxT_dbg = nc.dram_tensor("ctxT_dbg", [H, BS], BF,
                                  kind="ExternalOutput").ap()

    with tile.TileContext(nc) as tc:
        with (
            tc.tile_pool(name="const", bufs=1) as constp,
            tc.tile_pool(name="qkv", bufs=1) as qkvp,
            tc.tile_pool(name="ctile", bufs=12) as ctp,
        ):
            # ---- phase 0: constants (phase-1 critical ones first) ----
            wq_sb = constp.tile([128, JT, CI], BF)
            wk_sb = constp.tile([128, JT, CI], BF)
            wv_sb = constp.tile([128, JT, CI], BF)
            # wq on the fast gpsimd queue first; wk/wv are emitted inside
            # the chunk loop right after the first hs chunk so the queue
            # order is wq, hs0, wk, wv, hs1, ...
            nc.gpsimd.dma_start(wq_sb[:],
                                wqT.rearrange("(jt p) i -> p jt i", p=128))
            bq_sb = constp.tile([128, HPC], F32)
            bk_sb = constp.tile([128, HPC], F32)
            for b_sb, b_dr in ((bq_sb, bq), (bk_sb, bk)):
                for hi in range(HPC):
                    nc.scalar.dma_start(b_sb[:, hi:hi + 1],
                                        b_dr[hi * 128:(hi + 1) * 128, :])
            bv_sb = constp.tile([1, CI], BF)
            nc.scalar.dma_start(bv_sb[:], bv[:])
            alibi_sb = constp.tile([128, B * HPC, KT], F32)
            nc.scalar.dma_start(
                alibi_sb[:], alibi.rearrange("r (kt p) -> p r kt", p=128))
            ones_col_f32 = constp.tile([128, 1], F32)  # den lhsT (K=128, M=1)
            ones_row_bf = constp.tile([1, 128], BF)    # bias lhsT (K=1, M=128)
            ones_row_f32 = constp.tile([1, 128], F32)  # bcast lhsT (K=1, M=128)
            nc.vector.memset(ones_col_f32[:], 1.0)
            nc.vector.memset(ones_row_bf[:], 1.0)
            nc.vector.memset(ones_row_f32[:], 1.0)

            # persistent per-core activations
            qT_sb = qkvp.tile([128, HPC, BS], BF)      # [d, hi, ss]
            kT_sb = qkvp.tile([128, HPC, BS], BF)
            v_sb = qkvp.tile([128, BS // 128, CI], BF)  # [ss%128, ss//128, i]

            # ---- phase 1: QKV projections ----
            hsT_r = hsT.rearrange("(jt p) s -> p jt s", p=128)
            with (
                tc.tile_pool(name="hsb", bufs=3) as hsp,
                tc.tile_pool(name="p1psum", bufs=4,
                             space=bass.MemorySpace.PSUM) as p1p,
            ):
                for ch in range(N_CHUNKS):
                    s0 = ch * SS_CHUNK
                    hsb = hsp.tile([128, JT, SS_CHUNK], BF, name="hsb")
                    nc.gpsimd.dma_start(hsb[:], hsT_r[:, :, s0:s0 + SS_CHUNK])
                    if ch == 0:
                        nc.gpsimd.dma_start(
                            wk_sb[:],
                            wkT.rearrange("(jt p) i -> p jt i", p=128))
                        nc.gpsimd.dma_start(
                            wv_sb[:],
                            wvT.rearrange("(jt p) i -> p jt i", p=128))
                    for w_sb, b_col, o_sb, scale in (
                        (wq_sb, bq_sb, qT_sb, INV_NORM),
                        (wk_sb, bk_sb, kT_sb, 1.0),
                    ):
                        for hi in range(HPC):
                            ps = p1p.tile([128, SS_CHUNK], F32, name="ps_qk")
                            for jt in range(JT):
                                nc.tensor.matmul(
                                    ps[:],
                                    w_sb[:, jt, hi * 128:(hi + 1) * 128],
                                    hsb[:, jt, :],
                                    start=(jt == 0), stop=(jt == JT - 1))
                            nc.scalar.activation(
                                o_sb[:, hi, s0:s0 + SS_CHUNK], ps[:],
                                mybir.ActivationFunctionType.Identity,
                                bias=b_col[:, hi:hi + 1], scale=scale)
                    for st in range(SS_CHUNK // 128):
                        ps = p1p.tile([128, CI], F32, name="ps_v")
                        nc.tensor.matmul(ps[:], ones_row_bf[:], bv_sb[:],
                                         start=True, stop=False)
                        for jt in range(JT):
                            nc.tensor.matmul(
                                ps[:],
                                hsb[:, jt, st * 128:(st + 1) * 128],
                                wv_sb[:, jt, :],
                                start=False, stop=(jt == JT - 1))
                        nc.scalar.copy(v_sb[:, ch * 4 + st, :], ps[:])

            # late consts (dense phase only) — declared after phase 1 so
            # their DMAs don't delay the first projections
            wd_sb = constp.tile([128, IT, CI], BF)
            nc.sync.dma_start(
                wd_sb[:], wdT.rearrange("(jt p) i -> p jt i", p=128))
            bd_col = constp.tile([128, HPC], F32)
            for ci in range(HPC):
                nc.sync.dma_start(bd_col[:, ci:ci + 1],
                                  bd_f32[ci * 128:(ci + 1) * 128, :])

            # ---- phase 2+3: attention blocks + chunked AllGather ----
            with (
                tc.tile_pool(name="stp", bufs=3,
                             space=bass.MemorySpace.PSUM) as stp,
                tc.tile_pool(name="ptp", bufs=12) as ptp,
                tc.tile_pool(name="accp", bufs=1,
                             space=bass.MemorySpace.PSUM) as accp,
                tc.tile_pool(name="normp", bufs=2) as normp,
            ):
                LAG = 6
                pending_tail = [None]

                def flush_tail():
                    if pending_tail[0] is not None:
                        pending_tail[0]()
                        pending_tail[0] = None

                for blk in range(N_BLOCKS):
                    b, qh = divmod(blk, N_BLOCKS // B)
                    q0 = b * S + qh * QBLK
                    for hi in range(HPC):
                        bh = b * HPC + hi
                        ctx_ps = accp.tile([128, QBLK], F32, name="ctx_ps")
                        acc_sb = normp.tile([128, QBLK], F32, name="acc_sb")
                        pts = []

                        def consume(kt, ctx_ps=ctx_ps, acc_sb=acc_sb,
                                    pts=pts, b=b, hi=hi):
                            pt = pts[kt]
                            for half in range(2):
                                hs_ = slice(half * SS_CHUNK,
                                            (half + 1) * SS_CHUNK)
                                nc.tensor.matmul(
                                    ctx_ps[:, hs_],
                                    v_sb[:, (b * S) // 128 + kt,
                                         hi * 128:(hi + 1) * 128],
                                    pt[:, half, :],
                                    start=(kt == 0), stop=(kt == KT - 1))
                            # denominator partial sums on DVE (off PE):
                            # bf16 pair-sum (2x DVE rate), f32 chain
                            if kt % 2 == 1:
                                pa = pts[kt - 1][:].rearrange(
                                    "p a b -> p (a b)")
                                pb = pt[:].rearrange("p a b -> p (a b)")
                                psum2 = normp.tile([128, QBLK], BF,
                                                   name="psum2")
                                nc.vector.tensor_add(psum2[:], pa, pb)
                                if kt == 1:
                                    nc.vector.tensor_copy(acc_sb[:],
                                                          psum2[:])
                                else:
                                    nc.vector.tensor_add(acc_sb[:],
                                                         acc_sb[:],
                                                         psum2[:])

                        for kt in range(KT):
                            k0 = b * S + kt * 128
                            st_ps = stp.tile([128, 2, SS_CHUNK], F32,
                                             name="st_ps")
                            for half in range(2):
                                nc.tensor.matmul(
                                    st_ps[:, half, :],
                                    kT_sb[:, hi, k0:k0 + 128],
                                    qT_sb[:, hi,
                                          q0 + half * SS_CHUNK:
                                          q0 + (half + 1) * SS_CHUNK],
                                    start=True, stop=True)
                            pt = ptp.tile([128, 2, SS_CHUNK], BF, name="pt")
                            # q pre-scaled by INV_NORM in phase 1; alibi is
                            # a per-partition (key-position) bias
                            nc.scalar.activation(
                                pt[:], st_ps[:],
                                mybir.ActivationFunctionType.Exp,
                                bias=alibi_sb[:, bh, kt:kt + 1])
                            pts.append(pt)
                            # previous group's normalize tail slots in
                            # behind our first few St/exp emissions
                            if kt == 2:
                                flush_tail()
                            if kt >= LAG:
                                consume(kt - LAG)
                        for kt in range(KT - LAG, KT):
                            consume(kt)
                        # cross-partition reduce of acc -> den (borrows an
                        # stp slot; acc chain finishes under the last ctx
                        # matmuls)
                        den_ps = stp.tile([128, 2, SS_CHUNK], F32,
                                          name="st_ps")
                        for half in range(2):
                            nc.tensor.matmul(
                                den_ps[:1, half, :], ones_col_f32[:],
                                acc_sb[:, half * SS_CHUNK:
                                       (half + 1) * SS_CHUNK],
                                start=True, stop=True)
                        den_sb = normp.tile([1, QBLK], F32, name="den_sb")
                        # ACT is idle at the group boundary; DVE is draining
                        # the pair-sum chain — use ACT for this copy
                        nc.scalar.copy(
                            den_sb[:],
                            den_ps[:1, :, :].rearrange("p a b -> p (a b)"))

                        def tail(ctx_ps=ctx_ps, den_sb=den_sb, blk=blk,
                                 hi=hi):
                            denb_ps = stp.tile([128, 2, SS_CHUNK], F32,
                                               name="st_ps")
                            for half in range(2):
                                nc.tensor.matmul(
                                    denb_ps[:, half, :], ones_row_f32[:],
                                    den_sb[:, half * SS_CHUNK:
                                           (half + 1) * SS_CHUNK],
                                    start=True, stop=True)
                            denb_sb = normp.tile([128, QBLK], F32,
                                                 name="denb_sb")
                            nc.vector.reciprocal_approx_fast(
                                denb_sb[:],
                                denb_ps[:].rearrange("p a b -> p (a b)"))
                            ctxn_sb = normp.tile([128, QBLK], BF,
                                                 name="ctxn_sb")
                            nc.vector.tensor_mul(ctxn_sb[:], ctx_ps[:],
                                                 denb_sb[:])
                            nc.sync.dma_start(bounce[blk, hi], ctxn_sb[:])
                            nc.gpsimd.collective_compute(
                                "AllGather", mybir.AluOpType.bypass,
                                replica_groups=[list(range(N_CORES))],
                                ins=[bounce[blk, hi]],
                                outs=[gath[hi, blk]])

                        pending_tail[0] = tail
                flush_tail()

            if DEBUG_OUTPUTS:
                nc.sync.dma_start(qT_dbg[:],
                                  qT_sb[:].rearrange("p a b -> p (a b)"))
                nc.sync.dma_start(kT_dbg[:],
                                  kT_sb[:].rearrange("p a b -> p (a b)"))
                nc.sync.dma_start(v_dbg[:],
                                  v_sb[:].rearrange("p a b -> p (a b)"))
                dbg_r = ctxT_dbg.rearrange("(c x d) s -> c x d s", x=HPC,
                                           d=128)
                for blk in range(N_BLOCKS):
                    b, qh = divmod(blk, N_BLOCKS // B)
                    q0 = b * S + qh * QBLK
                    for hi in range(HPC):
                        nc.sync.dma_start(
                            dbg_r[:, hi, :, q0:q0 + QBLK],
                            gath[hi, blk].rearrange("(c d) s -> c d s",
                                                    d=128))

            # ---- phase 4: output projection (out^T form: Wd stationary,
            # LDWEIGHTS amortized over the moving ctx^T) + bias + residual
            with (
                tc.tile_pool(name="dpsum", bufs=8,
                             space=bass.MemorySpace.PSUM) as dpp,
                tc.tile_pool(name="outp", bufs=4) as outp,
            ):
                NSC = QBLK // SS_CHUNK      # 2 seq chunks per block
                for blk in range(N_BLOCKS):
                    b, qh = divmod(blk, N_BLOCKS // B)
                    q0 = b * S + qh * QBLK
                    dps = [dpp.tile([128, SS_CHUNK], F32, name="dps")
                           for _ in range(HPC * NSC)]
                    # hi=0 rows (even it) first: their AllGather chunk
                    # lands one attention group earlier than hi=1's
                    it_order = [*range(0, IT, 2), *range(1, IT, 2)]
                    for j, it in enumerate(it_order):
                        ctile = ctp.tile([128, QBLK], BF, name="ctile")
                        # scalar queue is idle after the constant loads, so
                        # these prefetch during phase 2 as AG chunks land
                        nc.scalar.dma_start(
                            ctile[:],
                            gath[it % HPC, blk,
                                 (it // HPC) * 128:(it // HPC + 1) * 128, :])
                        for ct in range(HPC):
                            for sc in range(NSC):
                                nc.tensor.matmul(
                                    dps[ct * NSC + sc][:],
                                    wd_sb[:, it, ct * 128:(ct + 1) * 128],
                                    ctile[:, sc * SS_CHUNK:
                                          (sc + 1) * SS_CHUNK],
                                    start=(j == 0), stop=(j == IT - 1))
                    for ct in range(HPC):
                        for sc in range(NSC):
                            c0 = ct * 128
                            s0_ = q0 + sc * SS_CHUNK
                            rtile = outp.tile([128, SS_CHUNK], F32,
                                              name="rtile")
                            nc.sync.dma_start(
                                rtile[:],
                                residT[c0:c0 + 128, s0_:s0_ + SS_CHUNK])
                            # bias is per-partition (output channel) here
                            osb = outp.tile([128, SS_CHUNK], F32,
                                            name="osb")
                            nc.scalar.activation(
                                osb[:], dps[ct * NSC + sc][:],
                                mybir.ActivationFunctionType.Identity,
                                bias=bd_col[:, ct:ct + 1])
                            osb2 = outp.tile([128, SS_CHUNK], F32,
                                             name="osb2")
                            nc.vector.tensor_add(osb2[:], osb[:], rtile[:])
                            nc.sync.dma_start(
                                outT[c0:c0 + 128, s0_:s0_ + SS_CHUNK],
                                osb2[:])

    nc.compile()
    return nc


_NC = None


def _get_nc():
    global _NC
    if _NC is None:
        _NC = _build()
    return _NC


def _prep_in_maps(hidden_states, residual, alibi, Wq, bq, Wk, bk, Wv, bv,
                  Wd, bd):
    hs = np.ascontiguousarray(np.asarray(hidden_states, np.float32)
                              .reshape(BS, H))
    hsT_bf = np.ascontiguousarray(hs.T).astype(BF16)
    resid = np.asarray(residual, np.float32).reshape(BS, H)
    alibi_r = np.asarray(alibi, np.float32).reshape(B, NH, S)
    in_maps = []
    for c in range(N_CORES):
        sl = slice(c * CI, (c + 1) * CI)
        # alibi rows ordered (b, hi) to match kernel indexing bh = b*HPC+hi
        al = np.ascontiguousarray(
            alibi_r[:, c * HPC:(c + 1) * HPC, :].reshape(B * HPC, S))
        in_maps.append({
            "hsT": hsT_bf,
            "wqT": np.ascontiguousarray(np.asarray(Wq, np.float32)[sl].T)
                     .astype(BF16),
            "wkT": np.ascontiguousarray(np.asarray(Wk, np.float32)[sl].T)
                     .astype(BF16),
            "wvT": np.ascontiguousarray(np.asarray(Wv, np.float32)[sl].T)
                     .astype(BF16),
            "wdT": np.ascontiguousarray(np.asarray(Wd, np.float32)[sl].T)
                     .astype(BF16),
            "bq": np.asarray(bq, np.float32)[sl].reshape(CI, 1),
            "bk": np.asarray(bk, np.float32)[sl].reshape(CI, 1),
            "bv": np.asarray(bv, np.float32)[sl].reshape(1, CI).astype(BF16),
            "bd": np.asarray(bd, np.float32)[sl].reshape(CI, 1),
            "alibi": al,
            "residT": np.ascontiguousarray(resid[:, sl].T),
        })
    return in_maps


def run(trace=False, trace_cores=None, stitch_traces=False, **inputs):
    nc = _get_nc()
    in_maps = _prep_in_maps(**inputs)
    res = bass_utils.run_bass_kernel_spmd(
        nc, in_maps, core_ids=list(range(N_CORES)), trace=trace,
        trace_cores=trace_cores, stitch_traces=stitch_traces)
    full = np.empty((BS, H), np.float32)
    for c in range(N_CORES):
        full[:, c * CI:(c + 1) * CI] = res.results[c]["outT"].T
    return full.reshape(B, S, H), res


def kernel(**inputs):
    out, _ = run(trace=False, **inputs)
    return out
